# revision 9
# baseline (speedup 1.0000x reference)
"""AnomalyAttention on 8 Trainium2 NeuronCores (Bass/Tile), data-parallel over batch.

Problem: B,L,H,E = 8,1024,8,64
  score  = (1/sqrt(E)) * einsum('blhe,bshe->bhls', Q, K)
  gauss  = kappa/sig_l * exp(-(l-s)^2 / (2 sig_l^2))       (kappa = 1/sqrt(2 pi))
  G_V    = softmax(score, s) @ G_values
  L_V    = softmax(score + gauss, s) @ L_values

Device strategy (per core = one batch element, loop over 8 heads):
  Work in the transposed layout P[s, l] so the attention@V matmuls need no
  transposes.  With c_l = gauss[l,l] (peak), exp(score+gauss-c_l) =
  P * exp(gauss-c_l) where P = exp(score) is shared with the Global path and
  exp(gauss-c_l) equals the constant encg_l = exp(-c_l) except in a +-16 band
  around the diagonal.  So:
    L_num   = encg_l * (Vl^T P) + Vl^T (P .* W)     with W = exp(gauss-c)-encg
    L_den   = encg_l * Z + ones^T (P .* W),  Z = ones^T P
  The device computes  Vg^T P | Vl^T P (one fused M=128 matmul chain),
  Vl^T (P.*W) (banded, with a ones column for the band Z row), and Z = 1^T P.

  Engine balance: the L*L exp is split between ScalarE (true exp, ~2/3 of
  chunks) and VectorE (single-op Schraudolph fast-exp: i16 = round(a*score+b)
  bit-viewed as bf16, +-3% rel err on 3/8 of chunks).  Z uses one persistent
  PSUM bank with one-hot M=16 weights per (head, half) in 4 col-group-packed
  streams, so only ONE final Z copy/DMA is needed.  Band multiplies cover both
  heads of a pair per instruction ([128, 2, len] APs).  PSUM evacuation is
  split: GL tiles on VectorE, B tiles on ScalarE.
"""

import math
import numpy as np
import ml_dtypes

BF16 = ml_dtypes.bfloat16
B, L, H, E = 8, 1024, 8, 64
NCH = L // 128          # 8 s-chunks of 128
BAND = 16               # gauss band halfwidth (W < 3e-7 beyond; bf16-invisible)
WW = 128 + 2 * BAND     # 160: W tile width in l per s-chunk
N_CORES = 8
DVE_KS = (1, 3, 5)      # s-chunks whose exp runs on VectorE (fast-exp)

LOG2E = 1.4426950408889634
FX_SIGMA = 0.0579       # Schraudolph minimax centering
FX_A = 128.0 * LOG2E / math.sqrt(E)       # folds the 1/sqrt(E) score scale
FX_B = (127.0 - FX_SIGMA) * 128.0

_NC_CACHE = {}


def _build_nc():
    if "nc" in _NC_CACHE:
        return _NC_CACHE["nc"]
    import concourse.bacc as bacc
    import concourse.tile as tile
    from concourse import mybir
    from concourse.tile import add_dep_helper

    f32 = mybir.dt.float32
    bf16 = mybir.dt.bfloat16
    i16 = mybir.dt.int16

    nc = bacc.Bacc()
    qkt_d = nc.declare_dram_parameter("qkt", [4, 128, 2 * L], bf16, isOutput=False)
    # vgl[h, :, k, 0:64] = V_g chunk, 64:128 = V_l chunk, col 128 = ones
    vgl_d = nc.declare_dram_parameter("vgl", [H, 128, NCH, 129], bf16, isOutput=False)
    # wband2[i, :, k, p, :]: gauss band W for head 2i+p
    wbd_d = nc.declare_dram_parameter("wband", [4, 128, NCH, 2, WW], bf16,
                                      isOutput=False)
    # out[h, half, 0] = [128, 512]: rows 0:64 Vg^T P, rows 64:128 Vl^T P
    # out[h, half, 1] = [65, 512]: band correction (+ band Z row 64)
    outGL_d = nc.declare_dram_parameter("outGL", [H, 2, 128, 512], bf16, isOutput=True)
    outB_d = nc.declare_dram_parameter("outB", [H, 2, 65, 512], bf16, isOutput=True)
    # one-hot Z accumulator: row 32j+m = partial Z for (h,half)=m over chunks
    # k == j (mod 4); host sums the 4 j-streams.
    outZ_d = nc.declare_dram_parameter("outZ", [128, 512], f32, isOutput=True)

    with tile.TileContext(nc) as tc:
        with (
            tc.tile_pool(name="ones_p", bufs=1) as ones_p,
            tc.tile_pool(name="qkt_p", bufs=2) as qkt_p,
            tc.tile_pool(name="v_p", bufs=4) as v_p,
            tc.tile_pool(name="w_p", bufs=2) as w_p,
            tc.tile_pool(name="pg_p", bufs=20) as pg_p,
            tc.tile_pool(name="mb_p", bufs=34) as mb_p,
            tc.tile_pool(name="stg_p", bufs=3) as stg_p,
            tc.tile_pool(name="sc_p", bufs=2, space="PSUM") as sc_p,
            tc.tile_pool(name="acc_p", bufs=1, space="PSUM") as acc_p,
        ):
            # sliding one-hot weight bank for Z: col 15 is ones; variant m
            # (= 4i+2p+half) is zw[:, 15-m:31-m]
            zw = ones_p.tile([128, 31], bf16, tag="zw", bufs=1)
            nc.vector.memset(zw, 0.0)
            nc.vector.memset(zw[:, 15:16], 1.0)
            zrow = ones_p.tile([1, 512], bf16, tag="zrow", bufs=1)
            nc.vector.memset(zrow, 0.0)
            # one persistent Z bank for the whole kernel; cleared once, all Z
            # matmuls start=False; single final copy+DMA
            accZ = acc_p.tile([128, 512], f32, tag="accZ", bufs=1)
            nc.tensor.matmul(out=accZ, lhsT=zrow[:, 0:128], rhs=zrow,
                             start=True, stop=False, skip_group_check=True)
            # PE warm-up: zero matmuls into a scratch bank during the initial
            # input-DMA wait, so the HAM un-throttles (1.2 -> 2.4 GHz) sooner
            # and the first real QK matmuls don't run cold.
            scr = acc_p.tile([128, 512], f32, tag="scr", bufs=1)
            for _ in range(2):
                nc.tensor.matmul(out=scr, lhsT=zrow[:, 0:128], rhs=zrow,
                                 start=True, stop=True, skip_group_check=True)

            state = {}

            def emit_sweep_half(i, q):
                if q == 0:
                    qt = qkt_p.tile([128, 2 * L], bf16, tag="qkt", bufs=2,
                                    name="qt")
                    # DMA in need-order: keys-lo + queries-q0 unblock the
                    # first QK chunks; keys-hi next; queries-q1 in half 1.
                    nc.sync.dma_start(out=qt[:, 1024:1536],
                                      in_=qkt_d.ap()[i][:, 1024:1536])
                    nc.sync.dma_start(out=qt[:, 0:512],
                                      in_=qkt_d.ap()[i][:, 0:512])
                    nc.sync.dma_start(out=qt[:, 1536:2048],
                                      in_=qkt_d.ap()[i][:, 1536:2048])
                    nc.sync.dma_start(out=qt[:, 512:1024],
                                      in_=qkt_d.ap()[i][:, 512:1024])
                    wb = w_p.tile([128, NCH, 2, WW], bf16, tag="wb", bufs=2,
                                  name="wb")
                    # issue from GpSimd so descriptor setup doesn't serialize
                    # behind the qkt DMAs on the Sync engine
                    nc.gpsimd.dma_start(out=wb, in_=wbd_d.ap()[i])
                    state[i] = {
                        "qt": qt, "wb": wb, "mb": [None] * NCH,
                        "pgq": [[None, None] for _ in range(NCH)],
                        "vgls": [],
                    }
                else:
                    qt = state[i]["qt"]
                    wb = state[i]["wb"]
                st = state[i]
                # QK with both heads in lockstep: per (chunk k, l-half q), one
                # [128, 2, 512] PSUM tile holds head-e scores in [:,0,:] and
                # head-o scores in [:,1,:]; the two matmuls use disjoint PE row
                # groups.  exp: ScalarE true exp or VectorE fast-exp per chunk.
                for k in range(NCH):
                    sc = sc_p.tile([128, 2, 512], f32, tag="sc", bufs=2,
                                   name="sc")
                    for p in range(2):
                        pslc = slice(64 * p, 64 * p + 64)
                        nc.tensor.matmul(
                            out=sc[:, p, :],
                            lhsT=qt[pslc, L + 128 * k:L + 128 * (k + 1)],
                            rhs=qt[pslc, 512 * q:512 * (q + 1)],
                            start=True, stop=True,
                            tile_position=(64 * p, 0),
                        )
                    pgk = pg_p.tile([128, 2, 512], bf16, tag="pg", bufs=40,
                                    name="pgk")
                    if k in DVE_KS:
                        nc.vector.tensor_scalar(
                            out=pgk.bitcast(i16), in0=sc,
                            scalar1=FX_A, scalar2=FX_B,
                            op0=mybir.AluOpType.mult,
                            op1=mybir.AluOpType.add,
                        )
                    else:
                        nc.scalar.activation(
                            out=pgk, in_=sc,
                            func=mybir.ActivationFunctionType.Exp,
                            scale=1.0 / math.sqrt(E),
                        )
                    st["pgq"][k][q] = pgk
                    # band product piece for this l-half (both heads at once)
                    a0 = max(0, 128 * k - BAND)
                    b0 = min(L, 128 * k + 128 + BAND)
                    a = max(a0, 512 * q)
                    bb = min(b0, 512 * (q + 1))
                    if bb > a:
                        if st["mb"][k] is None:
                            st["mb"][k] = mb_p.tile([128, 2, WW], bf16,
                                                    tag="mb", bufs=24,
                                                    name="mbk")
                        woff = a - (128 * k - BAND)
                        nc.vector.tensor_mul(
                            out=st["mb"][k][:, :, woff:woff + (bb - a)],
                            in0=pgk[:, :, a - 512 * q:bb - 512 * q],
                            in1=wb[:, k, :, woff:woff + (bb - a)],
                        )
                if q == 0:
                    for p in range(2):
                        h = 2 * i + p
                        vgl = v_p.tile([128, NCH, 129], bf16, tag=f"vgl{p}",
                                       bufs=2, name="vgl")
                        nc.gpsimd.dma_start(out=vgl, in_=vgl_d.ap()[h])
                        st["vgls"].append(vgl)

            def emit_av_half(i, half):
                st = state[i]
                pgq, mb = st["pgq"], st["mb"]
                for p in range(2):
                    h = 2 * i + p
                    vgl = st["vgls"][p]
                    m = 4 * i + 2 * p + half
                    h0 = half * 512
                    accGL = acc_p.tile([128, 512], f32, tag="accGL", bufs=1,
                                       name="accGL")
                    accB = acc_p.tile([65, 512], f32, tag="accB", bufs=1,
                                      name="accB")
                    for k in range(NCH):
                        rhs = pgq[k][half][:, p, :]
                        nc.tensor.matmul(out=accGL, lhsT=vgl[:, k, 0:128],
                                         rhs=rhs,
                                         start=(k == 0), stop=(k == NCH - 1))
                    for k in range(NCH):
                        rhs = pgq[k][half][:, p, :]
                        j = k % 4
                        nc.tensor.matmul(out=accZ[32 * j:32 * j + 16, :],
                                         lhsT=zw[:, 15 - m:31 - m], rhs=rhs,
                                         start=False, stop=False,
                                         tile_position=(0, 32 * j),
                                         skip_group_check=True)
                    spans = []
                    for k in range(NCH):
                        a = max(0, 128 * k - BAND, h0)
                        bb = min(L, 128 * k + 128 + BAND, h0 + 512)
                        if bb > a:
                            spans.append((k, a, bb))
                    b_first = None
                    for jj, (k, a, bb) in enumerate(spans):
                        off = a - (128 * k - BAND)
                        mmb = nc.tensor.matmul(
                            out=accB[:, a - h0:bb - h0],
                            lhsT=vgl[:, k, 64:129],
                            rhs=mb[k][:, p, off:off + (bb - a)],
                            start=(jj == 0), stop=(jj == len(spans) - 1),
                            skip_group_check=True,
                        )
                        if jj == 0:
                            b_first = mmb
                        else:
                            add_dep_helper(mmb.ins, b_first.ins,
                                           reason="bank clear first")
                    stgGL = stg_p.tile([128, 512], bf16, tag="stgGL", bufs=3,
                                       name="stgGL")
                    nc.vector.tensor_copy(out=stgGL, in_=accGL)
                    nc.sync.dma_start(out=outGL_d.ap()[h, half], in_=stgGL)
                    stgB = stg_p.tile([65, 512], bf16, tag="stgB", bufs=3,
                                      name="stgB")
                    nc.scalar.copy(out=stgB, in_=accB)
                    nc.sync.dma_start(out=outB_d.ap()[h, half], in_=stgB)

            # software pipeline at half-sweep granularity: AV blocks for
            # (pair, half) are emitted one half-step behind the QK/exp sweep
            # so the final PE-only tail is a single av-half, not a full pair.
            steps = []
            for i in range(4):
                steps.append(("S", i, 0))
                steps.append(("S", i, 1))
            sched = []
            av_q = []
            for kind, i, q in steps:
                sched.append(("S", i, q))
                av_q.append(("A", i, q))
                if len(av_q) >= 2:
                    sched.append(av_q.pop(0))
            sched.extend(av_q)
            for kind, i, q in sched:
                if kind == "S":
                    emit_sweep_half(i, q)
                else:
                    emit_av_half(i, q)
            # single Z evacuation at the very end
            stgZ = stg_p.tile([128, 512], f32, tag="stgZ", bufs=1, name="stgZ")
            nc.vector.tensor_copy(out=stgZ, in_=accZ)
            nc.sync.dma_start(out=outZ_d.ap(), in_=stgZ)
    nc.compile()
    _NC_CACHE["nc"] = nc
    return nc


def _host_prep(G_queries, G_keys, G_values, L_values, sigma):
    """Build per-core input dicts + host-side encg [L, H] per core."""
    inv_sqrt_2pi = 1.0 / math.sqrt(2.0 * math.pi)
    sig = sigma.astype(np.float32)
    sig = 1.0 / (1.0 + np.exp(-5.0 * sig.astype(np.float64)))
    sig = (sig + 1e-05).astype(np.float32)
    sig = (np.float32(3.0) ** sig) - np.float32(1.0)          # [B, L, H]
    c = inv_sqrt_2pi / sig.astype(np.float64)                  # [B, L, H]
    encg = np.exp(-c)                                          # [B, L, H]
    nhi = 1.0 / (2.0 * sig.astype(np.float64) ** 2)

    in_maps = []
    aux = []
    for b in range(B):
        qkt = np.empty((4, 128, 2 * L), BF16)
        for h in range(H):
            i, p = divmod(h, 2)
            qkt[i, 64 * p:64 * p + 64, :L] = G_queries[b, :, h, :].T
            qkt[i, 64 * p:64 * p + 64, L:] = G_keys[b, :, h, :].T
        # packed V: [V_g | V_l | ones] per chunk, layout [H, 128, NCH, 129]
        vgl = np.empty((H, 128, NCH, 129), BF16)
        gv = G_values[b].reshape(NCH, 128, H, E)   # [k, p, h, e]
        lv = L_values[b].reshape(NCH, 128, H, E)
        vgl[:, :, :, 0:64] = np.ascontiguousarray(gv.transpose(2, 1, 0, 3))
        vgl[:, :, :, 64:128] = np.ascontiguousarray(lv.transpose(2, 1, 0, 3))
        vgl[..., 128] = 1.0
        # W band tiles [4, 128, NCH, 2, WW] (pairs of heads)
        wband = np.zeros((4, 128, NCH, 2, WW), BF16)
        s_off = np.arange(128)
        j_off = np.arange(WW)
        for k in range(NCH):
            s_idx = 128 * k + s_off                  # [128]
            l_idx = 128 * k - BAND + j_off           # [WW]
            valid = (l_idx >= 0) & (l_idx < L)
            lvx = np.clip(l_idx, 0, L - 1)
            d = l_idx[None, :] - s_idx[:, None]      # [128, WW]
            band_ok = (np.abs(d) <= BAND) & valid[None, :]
            for h in range(H):
                i, p = divmod(h, 2)
                ch = c[b, lvx, h][None, :]
                g = ch * np.exp(-(d.astype(np.float64) ** 2) * nhi[b, lvx, h][None, :])
                W = np.exp(g - ch) - encg[b, lvx, h][None, :]
                W[~band_ok] = 0.0
                wband[i, :, k, p, :] = W.astype(np.float32)
        in_maps.append({"qkt": np.asarray(qkt),
                        "vgl": np.asarray(vgl),
                        "wband": np.asarray(wband)})
        aux.append(encg[b])  # [L, H]
    return in_maps, aux


def _host_post(outs, aux):
    G_V = np.empty((B, L, H, E), np.float32)
    L_V = np.empty((B, L, H, E), np.float32)
    for b in range(B):
        oGL = outs[b]["outGL"].astype(np.float64)  # [H, 2, 128, 512]
        oB = outs[b]["outB"].astype(np.float64)    # [H, 2, 65, 512]
        oZ = outs[b]["outZ"].astype(np.float64)    # [128, 512] one-hot packed
        # Z for (h, half) = m = 4i+2p+half: sum of the 4 col-group streams
        zm = oZ[0:16] + oZ[32:48] + oZ[64:80] + oZ[96:112]   # [16, 512]
        for h in range(H):
            i, p = divmod(h, 2)
            GLt = np.concatenate([oGL[h, 0], oGL[h, 1]], axis=1)  # [128, L]
            Bt = np.concatenate([oB[h, 0], oB[h, 1]], axis=1)     # [65, L]
            Z = np.concatenate([zm[4 * i + 2 * p + 0],
                                zm[4 * i + 2 * p + 1]])           # [L]
            e = aux[b][:, h]  # [L]
            G_V[b, :, h, :] = (GLt[0:64] / Z).T
            Lnum = GLt[64:128] * e[None, :] + Bt[:64]
            Lden = Z * e + Bt[64]
            L_V[b, :, h, :] = (Lnum / Lden).T
    return G_V, L_V


def kernel(G_queries, G_keys, G_values, L_values, sigma):
    from concourse.bass_utils import run_bass_kernel_spmd

    args = [np.asarray(x, dtype=np.float32) for x in
            (G_queries, G_keys, G_values, L_values, sigma)]
    nc = _build_nc()
    in_maps, aux = _host_prep(*args)
    res = run_bass_kernel_spmd(nc, in_maps, core_ids=list(range(N_CORES)),
                               trace=False)
    return _host_post(res.results, aux)


# revision 11
# speedup vs baseline: 1.0307x; 1.0307x over previous
"""AnomalyAttention on 8 Trainium2 NeuronCores (Bass/Tile), data-parallel over batch.

Problem: B,L,H,E = 8,1024,8,64
  score  = (1/sqrt(E)) * einsum('blhe,bshe->bhls', Q, K)
  gauss  = kappa/sig_l * exp(-(l-s)^2 / (2 sig_l^2))       (kappa = 1/sqrt(2 pi))
  G_V    = softmax(score, s) @ G_values
  L_V    = softmax(score + gauss, s) @ L_values

Device strategy (per core = one batch element, loop over 8 heads):
  Work in the transposed layout P[s, l] so the attention@V matmuls need no
  transposes.  With c_l = gauss[l,l] (peak), exp(score+gauss-c_l) =
  P * exp(gauss-c_l) where P = exp(score) is shared with the Global path and
  exp(gauss-c_l) equals the constant encg_l = exp(-c_l) except in a +-16 band
  around the diagonal.  So:
    L_num   = encg_l * (Vl^T P) + Vl^T (P .* W)     with W = exp(gauss-c)-encg
    L_den   = encg_l * Z + ones^T (P .* W),  Z = ones^T P
  The device computes  Vg^T P | Vl^T P (one fused M=128 matmul chain),
  Vl^T (P.*W) (banded, with a ones column for the band Z row), and Z = 1^T P.

  Engine balance: the L*L exp is split between ScalarE (true exp, ~2/3 of
  chunks) and VectorE (single-op Schraudolph fast-exp: i16 = round(a*score+b)
  bit-viewed as bf16, +-3% rel err on 3/8 of chunks).  Z uses one persistent
  PSUM bank with one-hot M=16 weights per (head, half) in 4 col-group-packed
  streams, so only ONE final Z copy/DMA is needed.  Band multiplies cover both
  heads of a pair per instruction ([128, 2, len] APs).  PSUM evacuation is
  split: GL tiles on VectorE, B tiles on ScalarE.
"""

import math
import numpy as np
import ml_dtypes

BF16 = ml_dtypes.bfloat16
B, L, H, E = 8, 1024, 8, 64
NCH = L // 128          # 8 s-chunks of 128
BAND = 16               # gauss band halfwidth (W < 3e-7 beyond; bf16-invisible)
WW = 128 + 2 * BAND     # 160: W tile width in l per s-chunk
N_CORES = 8
DVE_KS = (1, 3, 5)      # s-chunks whose exp runs on VectorE (fast-exp)

LOG2E = 1.4426950408889634
FX_SIGMA = 0.0579       # Schraudolph minimax centering
FX_A = 128.0 * LOG2E / math.sqrt(E)       # folds the 1/sqrt(E) score scale
FX_B = (127.0 - FX_SIGMA) * 128.0

_NC_CACHE = {}


def _build_nc():
    if "nc" in _NC_CACHE:
        return _NC_CACHE["nc"]
    import concourse.bacc as bacc
    import concourse.tile as tile
    from concourse import mybir
    from concourse.tile import add_dep_helper

    f32 = mybir.dt.float32
    bf16 = mybir.dt.bfloat16
    i16 = mybir.dt.int16

    nc = bacc.Bacc()
    qkt_d = nc.declare_dram_parameter("qkt", [4, 128, 2 * L], bf16, isOutput=False)
    # vgl[h, :, k, 0:64] = V_g chunk, 64:128 = V_l chunk, col 128 = ones
    vgl_d = nc.declare_dram_parameter("vgl", [H, 128, NCH, 129], bf16, isOutput=False)
    # wband2[i, :, k, p, :]: gauss band W for head 2i+p
    wbd_d = nc.declare_dram_parameter("wband", [4, 128, NCH, 2, WW], bf16,
                                      isOutput=False)
    # out[h, half, 0] = [128, 512]: rows 0:64 Vg^T P, rows 64:128 Vl^T P
    # out[h, half, 1] = [65, 512]: band correction (+ band Z row 64)
    outGL_d = nc.declare_dram_parameter("outGL", [H, 2, 128, 512], bf16, isOutput=True)
    outB_d = nc.declare_dram_parameter("outB", [H, 2, 65, 512], bf16, isOutput=True)
    # one-hot Z accumulator: row 32j+m = partial Z for (h,half)=m over chunks
    # k == j (mod 4); host sums the 4 j-streams.
    outZ_d = nc.declare_dram_parameter("outZ", [128, 512], f32, isOutput=True)

    with tile.TileContext(nc) as tc:
        with (
            tc.tile_pool(name="ones_p", bufs=1) as ones_p,
            tc.tile_pool(name="qkt_p", bufs=2) as qkt_p,
            tc.tile_pool(name="v_p", bufs=4) as v_p,
            tc.tile_pool(name="w_p", bufs=2) as w_p,
            tc.tile_pool(name="pg_p", bufs=20) as pg_p,
            tc.tile_pool(name="mb_p", bufs=34) as mb_p,
            tc.tile_pool(name="stg_p", bufs=3) as stg_p,
            tc.tile_pool(name="sc_p", bufs=2, space="PSUM") as sc_p,
            tc.tile_pool(name="acc_p", bufs=1, space="PSUM") as acc_p,
        ):
            # sliding one-hot weight bank for Z: col 15 is ones; variant m
            # (= 4i+2p+half) is zw[:, 15-m:31-m]
            zw = ones_p.tile([128, 31], bf16, tag="zw", bufs=1)
            nc.vector.memset(zw, 0.0)
            nc.vector.memset(zw[:, 15:16], 1.0)
            zrow = ones_p.tile([1, 512], bf16, tag="zrow", bufs=1)
            nc.vector.memset(zrow, 0.0)
            # one persistent Z bank for the whole kernel; cleared once, all Z
            # matmuls start=False; single final copy+DMA
            accZ = acc_p.tile([128, 512], f32, tag="accZ", bufs=1)
            nc.tensor.matmul(out=accZ, lhsT=zrow[:, 0:128], rhs=zrow,
                             start=True, stop=False, skip_group_check=True)

            state = {}

            def emit_sweep_half(i, q):
                if q == 0:
                    qt = qkt_p.tile([128, 2 * L], bf16, tag="qkt", bufs=2,
                                    name="qt")
                    # DMA in need-order: keys-lo + queries-q0 unblock the
                    # first QK chunks; keys-hi next; queries-q1 in half 1.
                    nc.sync.dma_start(out=qt[:, 1024:1536],
                                      in_=qkt_d.ap()[i][:, 1024:1536])
                    nc.sync.dma_start(out=qt[:, 0:512],
                                      in_=qkt_d.ap()[i][:, 0:512])
                    nc.sync.dma_start(out=qt[:, 1536:2048],
                                      in_=qkt_d.ap()[i][:, 1536:2048])
                    wb = w_p.tile([128, NCH, 2, WW], bf16, tag="wb", bufs=2,
                                  name="wb")
                    nc.scalar.dma_start(out=wb, in_=wbd_d.ap()[i])
                    state[i] = {
                        "qt": qt, "wb": wb, "mb": [None] * NCH,
                        "pgq": [[None, None] for _ in range(NCH)],
                        "vgls": [],
                    }
                else:
                    qt = state[i]["qt"]
                    wb = state[i]["wb"]
                    nc.sync.dma_start(out=qt[:, 512:1024],
                                      in_=qkt_d.ap()[i][:, 512:1024])
                st = state[i]
                # QK with both heads in lockstep: per (chunk k, l-half q), one
                # [128, 2, 512] PSUM tile holds head-e scores in [:,0,:] and
                # head-o scores in [:,1,:]; the two matmuls use disjoint PE row
                # groups.  exp: ScalarE true exp or VectorE fast-exp per chunk.
                for k in range(NCH):
                    sc = sc_p.tile([128, 2, 512], f32, tag="sc", bufs=2,
                                   name="sc")
                    for p in range(2):
                        pslc = slice(64 * p, 64 * p + 64)
                        nc.tensor.matmul(
                            out=sc[:, p, :],
                            lhsT=qt[pslc, L + 128 * k:L + 128 * (k + 1)],
                            rhs=qt[pslc, 512 * q:512 * (q + 1)],
                            start=True, stop=True,
                            tile_position=(64 * p, 0),
                        )
                    pgk = pg_p.tile([128, 2, 512], bf16, tag="pg", bufs=40,
                                    name="pgk")
                    if k in DVE_KS:
                        nc.vector.tensor_scalar(
                            out=pgk.bitcast(i16), in0=sc,
                            scalar1=FX_A, scalar2=FX_B,
                            op0=mybir.AluOpType.mult,
                            op1=mybir.AluOpType.add,
                        )
                    else:
                        nc.scalar.activation(
                            out=pgk, in_=sc,
                            func=mybir.ActivationFunctionType.Exp,
                            scale=1.0 / math.sqrt(E),
                        )
                    st["pgq"][k][q] = pgk
                    # band product piece for this l-half (both heads at once)
                    a0 = max(0, 128 * k - BAND)
                    b0 = min(L, 128 * k + 128 + BAND)
                    a = max(a0, 512 * q)
                    bb = min(b0, 512 * (q + 1))
                    if bb > a:
                        if st["mb"][k] is None:
                            st["mb"][k] = mb_p.tile([128, 2, WW], bf16,
                                                    tag="mb", bufs=24,
                                                    name="mbk")
                        woff = a - (128 * k - BAND)
                        nc.vector.tensor_mul(
                            out=st["mb"][k][:, :, woff:woff + (bb - a)],
                            in0=pgk[:, :, a - 512 * q:bb - 512 * q],
                            in1=wb[:, k, :, woff:woff + (bb - a)],
                        )
                if q == 0:
                    for p in range(2):
                        h = 2 * i + p
                        vgl = v_p.tile([128, NCH, 129], bf16, tag=f"vgl{p}",
                                       bufs=2, name="vgl")
                        nc.scalar.dma_start(out=vgl, in_=vgl_d.ap()[h])
                        st["vgls"].append(vgl)

            def emit_av_half(i, half):
                st = state[i]
                pgq, mb = st["pgq"], st["mb"]
                for p in range(2):
                    h = 2 * i + p
                    vgl = st["vgls"][p]
                    m = 4 * i + 2 * p + half
                    h0 = half * 512
                    accGL = acc_p.tile([128, 512], f32, tag="accGL", bufs=1,
                                       name="accGL")
                    accB = acc_p.tile([65, 512], f32, tag="accB", bufs=1,
                                      name="accB")
                    for k in range(NCH):
                        rhs = pgq[k][half][:, p, :]
                        nc.tensor.matmul(out=accGL, lhsT=vgl[:, k, 0:128],
                                         rhs=rhs,
                                         start=(k == 0), stop=(k == NCH - 1))
                    for k in range(NCH):
                        rhs = pgq[k][half][:, p, :]
                        j = k % 4
                        nc.tensor.matmul(out=accZ[32 * j:32 * j + 16, :],
                                         lhsT=zw[:, 15 - m:31 - m], rhs=rhs,
                                         start=False, stop=False,
                                         tile_position=(0, 32 * j),
                                         skip_group_check=True)
                    spans = []
                    for k in range(NCH):
                        a = max(0, 128 * k - BAND, h0)
                        bb = min(L, 128 * k + 128 + BAND, h0 + 512)
                        if bb > a:
                            spans.append((k, a, bb))
                    b_first = None
                    for jj, (k, a, bb) in enumerate(spans):
                        off = a - (128 * k - BAND)
                        mmb = nc.tensor.matmul(
                            out=accB[:, a - h0:bb - h0],
                            lhsT=vgl[:, k, 64:129],
                            rhs=mb[k][:, p, off:off + (bb - a)],
                            start=(jj == 0), stop=(jj == len(spans) - 1),
                            skip_group_check=True,
                        )
                        if jj == 0:
                            b_first = mmb
                        else:
                            add_dep_helper(mmb.ins, b_first.ins,
                                           reason="bank clear first")
                    stgGL = stg_p.tile([128, 512], bf16, tag="stgGL", bufs=3,
                                       name="stgGL")
                    nc.vector.tensor_copy(out=stgGL, in_=accGL)
                    nc.sync.dma_start(out=outGL_d.ap()[h, half], in_=stgGL)
                    stgB = stg_p.tile([65, 512], bf16, tag="stgB", bufs=3,
                                      name="stgB")
                    nc.scalar.copy(out=stgB, in_=accB)
                    nc.scalar.dma_start(out=outB_d.ap()[h, half], in_=stgB)

            # software pipeline at half-sweep granularity: AV blocks for
            # (pair, half) are emitted one half-step behind the QK/exp sweep
            # so the final PE-only tail is a single av-half, not a full pair.
            steps = []
            for i in range(4):
                steps.append(("S", i, 0))
                steps.append(("S", i, 1))
            sched = []
            av_q = []
            for kind, i, q in steps:
                sched.append(("S", i, q))
                av_q.append(("A", i, q))
                if len(av_q) >= 2:
                    sched.append(av_q.pop(0))
            sched.extend(av_q)
            for kind, i, q in sched:
                if kind == "S":
                    emit_sweep_half(i, q)
                else:
                    emit_av_half(i, q)
            # single Z evacuation at the very end
            stgZ = stg_p.tile([128, 512], f32, tag="stgZ", bufs=1, name="stgZ")
            nc.vector.tensor_copy(out=stgZ, in_=accZ)
            nc.sync.dma_start(out=outZ_d.ap(), in_=stgZ)
    nc.compile()
    _NC_CACHE["nc"] = nc
    return nc


def _host_prep(G_queries, G_keys, G_values, L_values, sigma):
    """Build per-core input dicts + host-side encg [L, H] per core."""
    inv_sqrt_2pi = 1.0 / math.sqrt(2.0 * math.pi)
    sig = sigma.astype(np.float32)
    sig = 1.0 / (1.0 + np.exp(-5.0 * sig.astype(np.float64)))
    sig = (sig + 1e-05).astype(np.float32)
    sig = (np.float32(3.0) ** sig) - np.float32(1.0)          # [B, L, H]
    c = inv_sqrt_2pi / sig.astype(np.float64)                  # [B, L, H]
    encg = np.exp(-c)                                          # [B, L, H]
    nhi = 1.0 / (2.0 * sig.astype(np.float64) ** 2)

    in_maps = []
    aux = []
    for b in range(B):
        qkt = np.empty((4, 128, 2 * L), BF16)
        for h in range(H):
            i, p = divmod(h, 2)
            qkt[i, 64 * p:64 * p + 64, :L] = G_queries[b, :, h, :].T
            qkt[i, 64 * p:64 * p + 64, L:] = G_keys[b, :, h, :].T
        # packed V: [V_g | V_l | ones] per chunk, layout [H, 128, NCH, 129]
        vgl = np.empty((H, 128, NCH, 129), BF16)
        gv = G_values[b].reshape(NCH, 128, H, E)   # [k, p, h, e]
        lv = L_values[b].reshape(NCH, 128, H, E)
        vgl[:, :, :, 0:64] = np.ascontiguousarray(gv.transpose(2, 1, 0, 3))
        vgl[:, :, :, 64:128] = np.ascontiguousarray(lv.transpose(2, 1, 0, 3))
        vgl[..., 128] = 1.0
        # W band tiles [4, 128, NCH, 2, WW] (pairs of heads)
        wband = np.zeros((4, 128, NCH, 2, WW), BF16)
        s_off = np.arange(128)
        j_off = np.arange(WW)
        for k in range(NCH):
            s_idx = 128 * k + s_off                  # [128]
            l_idx = 128 * k - BAND + j_off           # [WW]
            valid = (l_idx >= 0) & (l_idx < L)
            lvx = np.clip(l_idx, 0, L - 1)
            d = l_idx[None, :] - s_idx[:, None]      # [128, WW]
            band_ok = (np.abs(d) <= BAND) & valid[None, :]
            for h in range(H):
                i, p = divmod(h, 2)
                ch = c[b, lvx, h][None, :]
                g = ch * np.exp(-(d.astype(np.float64) ** 2) * nhi[b, lvx, h][None, :])
                W = np.exp(g - ch) - encg[b, lvx, h][None, :]
                W[~band_ok] = 0.0
                wband[i, :, k, p, :] = W.astype(np.float32)
        in_maps.append({"qkt": np.asarray(qkt),
                        "vgl": np.asarray(vgl),
                        "wband": np.asarray(wband)})
        aux.append(encg[b])  # [L, H]
    return in_maps, aux


def _host_post(outs, aux):
    G_V = np.empty((B, L, H, E), np.float32)
    L_V = np.empty((B, L, H, E), np.float32)
    for b in range(B):
        oGL = outs[b]["outGL"].astype(np.float64)  # [H, 2, 128, 512]
        oB = outs[b]["outB"].astype(np.float64)    # [H, 2, 65, 512]
        oZ = outs[b]["outZ"].astype(np.float64)    # [128, 512] one-hot packed
        # Z for (h, half) = m = 4i+2p+half: sum of the 4 col-group streams
        zm = oZ[0:16] + oZ[32:48] + oZ[64:80] + oZ[96:112]   # [16, 512]
        for h in range(H):
            i, p = divmod(h, 2)
            GLt = np.concatenate([oGL[h, 0], oGL[h, 1]], axis=1)  # [128, L]
            Bt = np.concatenate([oB[h, 0], oB[h, 1]], axis=1)     # [65, L]
            Z = np.concatenate([zm[4 * i + 2 * p + 0],
                                zm[4 * i + 2 * p + 1]])           # [L]
            e = aux[b][:, h]  # [L]
            G_V[b, :, h, :] = (GLt[0:64] / Z).T
            Lnum = GLt[64:128] * e[None, :] + Bt[:64]
            Lden = Z * e + Bt[64]
            L_V[b, :, h, :] = (Lnum / Lden).T
    return G_V, L_V


def kernel(G_queries, G_keys, G_values, L_values, sigma):
    from concourse.bass_utils import run_bass_kernel_spmd

    args = [np.asarray(x, dtype=np.float32) for x in
            (G_queries, G_keys, G_values, L_values, sigma)]
    nc = _build_nc()
    in_maps, aux = _host_prep(*args)
    res = run_bass_kernel_spmd(nc, in_maps, core_ids=list(range(N_CORES)),
                               trace=False)
    return _host_post(res.results, aux)


# revision 12
# speedup vs baseline: 1.0358x; 1.0050x over previous
"""AnomalyAttention on 8 Trainium2 NeuronCores (Bass/Tile), data-parallel over batch.

Problem: B,L,H,E = 8,1024,8,64
  score  = (1/sqrt(E)) * einsum('blhe,bshe->bhls', Q, K)
  gauss  = kappa/sig_l * exp(-(l-s)^2 / (2 sig_l^2))       (kappa = 1/sqrt(2 pi))
  G_V    = softmax(score, s) @ G_values
  L_V    = softmax(score + gauss, s) @ L_values

Device strategy (per core = one batch element, loop over 8 heads):
  Work in the transposed layout P[s, l] so the attention@V matmuls need no
  transposes.  With c_l = gauss[l,l] (peak), exp(score+gauss-c_l) =
  P * exp(gauss-c_l) where P = exp(score) is shared with the Global path and
  exp(gauss-c_l) equals the constant encg_l = exp(-c_l) except in a +-16 band
  around the diagonal.  So:
    L_num   = encg_l * (Vl^T P) + Vl^T (P .* W)     with W = exp(gauss-c)-encg
    L_den   = encg_l * Z + ones^T (P .* W),  Z = ones^T P
  The device computes  Vg^T P | Vl^T P (one fused M=128 matmul chain),
  Vl^T (P.*W) (banded, with a ones column for the band Z row), and Z = 1^T P.

  Engine balance: the L*L exp is split between ScalarE (true exp, ~2/3 of
  chunks) and VectorE (single-op Schraudolph fast-exp: i16 = round(a*score+b)
  bit-viewed as bf16, +-3% rel err on 3/8 of chunks).  Z uses one persistent
  PSUM bank with one-hot M=16 weights per (head, half) in 4 col-group-packed
  streams, so only ONE final Z copy/DMA is needed.  Band multiplies cover both
  heads of a pair per instruction ([128, 2, len] APs).  PSUM evacuation is
  split: GL tiles on VectorE, B tiles on ScalarE.
"""

import math
import numpy as np
import ml_dtypes

BF16 = ml_dtypes.bfloat16
B, L, H, E = 8, 1024, 8, 64
NCH = L // 128          # 8 s-chunks of 128
BAND = 16               # gauss band halfwidth (W < 3e-7 beyond; bf16-invisible)
WW = 128 + 2 * BAND     # 160: W tile width in l per s-chunk
N_CORES = 8
DVE_KS = (1, 3, 5)      # s-chunks whose exp runs on VectorE (fast-exp)

LOG2E = 1.4426950408889634
FX_SIGMA = 0.0579       # Schraudolph minimax centering
FX_A = 128.0 * LOG2E / math.sqrt(E)       # folds the 1/sqrt(E) score scale
FX_B = (127.0 - FX_SIGMA) * 128.0

_NC_CACHE = {}


def _build_nc():
    if "nc" in _NC_CACHE:
        return _NC_CACHE["nc"]
    import concourse.bacc as bacc
    import concourse.tile as tile
    from concourse import mybir
    from concourse.tile import add_dep_helper

    f32 = mybir.dt.float32
    bf16 = mybir.dt.bfloat16
    i16 = mybir.dt.int16

    nc = bacc.Bacc()
    qkt_d = nc.declare_dram_parameter("qkt", [4, 128, 2 * L], bf16, isOutput=False)
    # vgl[h, :, k, 0:64] = V_g chunk, 64:128 = V_l chunk, col 128 = ones
    vgl_d = nc.declare_dram_parameter("vgl", [H, 128, NCH, 129], bf16, isOutput=False)
    # wband2[i, :, k, p, :]: gauss band W for head 2i+p
    wbd_d = nc.declare_dram_parameter("wband", [4, 128, NCH, 2, WW], bf16,
                                      isOutput=False)
    # out[h, half, 0] = [128, 512]: rows 0:64 Vg^T P, rows 64:128 Vl^T P
    # out[h, half, 1] = [65, 512]: band correction (+ band Z row 64)
    outGL_d = nc.declare_dram_parameter("outGL", [H, 2, 128, 512], bf16, isOutput=True)
    outB_d = nc.declare_dram_parameter("outB", [H, 2, 65, 512], bf16, isOutput=True)
    # one-hot Z accumulator: row 32j+m = partial Z for (h,half)=m over chunks
    # k == j (mod 4); host sums the 4 j-streams.
    outZ_d = nc.declare_dram_parameter("outZ", [128, 512], f32, isOutput=True)

    with tile.TileContext(nc) as tc:
        with (
            tc.tile_pool(name="ones_p", bufs=1) as ones_p,
            tc.tile_pool(name="qkt_p", bufs=2) as qkt_p,
            tc.tile_pool(name="v_p", bufs=4) as v_p,
            tc.tile_pool(name="w_p", bufs=2) as w_p,
            tc.tile_pool(name="pg_p", bufs=20) as pg_p,
            tc.tile_pool(name="mb_p", bufs=34) as mb_p,
            tc.tile_pool(name="stg_p", bufs=3) as stg_p,
            tc.tile_pool(name="sc_p", bufs=2, space="PSUM") as sc_p,
            tc.tile_pool(name="acc_p", bufs=1, space="PSUM") as acc_p,
        ):
            # sliding one-hot weight bank for Z: col 15 is ones; variant m
            # (= 4i+2p+half) is zw[:, 15-m:31-m]
            zw = ones_p.tile([128, 31], bf16, tag="zw", bufs=1)
            nc.vector.memset(zw, 0.0)
            nc.vector.memset(zw[:, 15:16], 1.0)
            zrow = ones_p.tile([1, 512], bf16, tag="zrow", bufs=1)
            nc.vector.memset(zrow, 0.0)
            # one persistent Z bank for the whole kernel; cleared once, all Z
            # matmuls start=False; single final copy+DMA
            accZ = acc_p.tile([128, 512], f32, tag="accZ", bufs=1)
            nc.tensor.matmul(out=accZ, lhsT=zrow[:, 0:128], rhs=zrow,
                             start=True, stop=False, skip_group_check=True)

            state = {}

            def emit_sweep_half(i, q):
                if q == 0:
                    qt = qkt_p.tile([128, 2 * L], bf16, tag="qkt", bufs=2,
                                    name="qt")
                    # DMA in need-order: keys-lo + queries-q0 unblock the
                    # first QK chunks; keys-hi next; queries-q1 in half 1.
                    nc.sync.dma_start(out=qt[:, 1024:1536],
                                      in_=qkt_d.ap()[i][:, 1024:1536])
                    nc.sync.dma_start(out=qt[:, 0:512],
                                      in_=qkt_d.ap()[i][:, 0:512])
                    nc.sync.dma_start(out=qt[:, 1536:2048],
                                      in_=qkt_d.ap()[i][:, 1536:2048])
                    wb = w_p.tile([128, NCH, 2, WW], bf16, tag="wb", bufs=2,
                                  name="wb")
                    nc.scalar.dma_start(out=wb, in_=wbd_d.ap()[i])
                    state[i] = {
                        "qt": qt, "wb": wb, "mb": [None] * NCH,
                        "pgq": [[None, None] for _ in range(NCH)],
                        "vgls": [],
                    }
                else:
                    qt = state[i]["qt"]
                    wb = state[i]["wb"]
                    nc.sync.dma_start(out=qt[:, 512:1024],
                                      in_=qkt_d.ap()[i][:, 512:1024])
                st = state[i]
                # QK with both heads in lockstep: per (chunk k, l-half q), one
                # [128, 2, 512] PSUM tile holds head-e scores in [:,0,:] and
                # head-o scores in [:,1,:]; the two matmuls use disjoint PE row
                # groups.  exp: ScalarE true exp or VectorE fast-exp per chunk.
                for k in range(NCH):
                    sc = sc_p.tile([128, 2, 512], f32, tag="sc", bufs=2,
                                   name="sc")
                    for p in range(2):
                        pslc = slice(64 * p, 64 * p + 64)
                        nc.tensor.matmul(
                            out=sc[:, p, :],
                            lhsT=qt[pslc, L + 128 * k:L + 128 * (k + 1)],
                            rhs=qt[pslc, 512 * q:512 * (q + 1)],
                            start=True, stop=True,
                            tile_position=(64 * p, 0),
                        )
                    pgk = pg_p.tile([128, 2, 512], bf16, tag="pg", bufs=40,
                                    name="pgk")
                    if k in DVE_KS:
                        nc.vector.tensor_scalar(
                            out=pgk.bitcast(i16), in0=sc,
                            scalar1=FX_A, scalar2=FX_B,
                            op0=mybir.AluOpType.mult,
                            op1=mybir.AluOpType.add,
                        )
                    else:
                        nc.scalar.activation(
                            out=pgk, in_=sc,
                            func=mybir.ActivationFunctionType.Exp,
                            scale=1.0 / math.sqrt(E),
                        )
                    st["pgq"][k][q] = pgk
                    # band product piece for this l-half (both heads at once)
                    a0 = max(0, 128 * k - BAND)
                    b0 = min(L, 128 * k + 128 + BAND)
                    a = max(a0, 512 * q)
                    bb = min(b0, 512 * (q + 1))
                    if bb > a:
                        if st["mb"][k] is None:
                            st["mb"][k] = mb_p.tile([128, 2, WW], bf16,
                                                    tag="mb", bufs=24,
                                                    name="mbk")
                        woff = a - (128 * k - BAND)
                        nc.vector.tensor_mul(
                            out=st["mb"][k][:, :, woff:woff + (bb - a)],
                            in0=pgk[:, :, a - 512 * q:bb - 512 * q],
                            in1=wb[:, k, :, woff:woff + (bb - a)],
                        )
                if q == 0:
                    for p in range(2):
                        h = 2 * i + p
                        vgl = v_p.tile([128, NCH, 129], bf16, tag=f"vgl{p}",
                                       bufs=2, name="vgl")
                        nc.scalar.dma_start(out=vgl, in_=vgl_d.ap()[h])
                        st["vgls"].append(vgl)

            def emit_av_half(i, half):
                st = state[i]
                pgq, mb = st["pgq"], st["mb"]
                for p in range(2):
                    h = 2 * i + p
                    vgl = st["vgls"][p]
                    m = 4 * i + 2 * p + half
                    h0 = half * 512
                    accGL = acc_p.tile([128, 512], f32, tag="accGL", bufs=2,
                                       name="accGL")
                    accB = acc_p.tile([65, 512], f32, tag="accB", bufs=1,
                                      name="accB")
                    for k in range(NCH):
                        rhs = pgq[k][half][:, p, :]
                        nc.tensor.matmul(out=accGL, lhsT=vgl[:, k, 0:128],
                                         rhs=rhs,
                                         start=(k == 0), stop=(k == NCH - 1))
                    for k in range(NCH):
                        rhs = pgq[k][half][:, p, :]
                        j = k % 4
                        nc.tensor.matmul(out=accZ[32 * j:32 * j + 16, :],
                                         lhsT=zw[:, 15 - m:31 - m], rhs=rhs,
                                         start=False, stop=False,
                                         tile_position=(0, 32 * j),
                                         skip_group_check=True)
                    spans = []
                    for k in range(NCH):
                        a = max(0, 128 * k - BAND, h0)
                        bb = min(L, 128 * k + 128 + BAND, h0 + 512)
                        if bb > a:
                            spans.append((k, a, bb))
                    b_first = None
                    for jj, (k, a, bb) in enumerate(spans):
                        off = a - (128 * k - BAND)
                        mmb = nc.tensor.matmul(
                            out=accB[:, a - h0:bb - h0],
                            lhsT=vgl[:, k, 64:129],
                            rhs=mb[k][:, p, off:off + (bb - a)],
                            start=(jj == 0), stop=(jj == len(spans) - 1),
                            skip_group_check=True,
                        )
                        if jj == 0:
                            b_first = mmb
                        else:
                            add_dep_helper(mmb.ins, b_first.ins,
                                           reason="bank clear first")
                    stgGL = stg_p.tile([128, 512], bf16, tag="stgGL", bufs=3,
                                       name="stgGL")
                    nc.vector.tensor_copy(out=stgGL, in_=accGL)
                    nc.sync.dma_start(out=outGL_d.ap()[h, half], in_=stgGL)
                    stgB = stg_p.tile([65, 512], bf16, tag="stgB", bufs=3,
                                      name="stgB")
                    nc.scalar.copy(out=stgB, in_=accB)
                    nc.scalar.dma_start(out=outB_d.ap()[h, half], in_=stgB)

            # software pipeline at half-sweep granularity: AV blocks for
            # (pair, half) are emitted one half-step behind the QK/exp sweep
            # so the final PE-only tail is a single av-half, not a full pair.
            steps = []
            for i in range(4):
                steps.append(("S", i, 0))
                steps.append(("S", i, 1))
            sched = []
            av_q = []
            for kind, i, q in steps:
                sched.append(("S", i, q))
                av_q.append(("A", i, q))
                if len(av_q) >= 2:
                    sched.append(av_q.pop(0))
            sched.extend(av_q)
            for kind, i, q in sched:
                if kind == "S":
                    emit_sweep_half(i, q)
                else:
                    emit_av_half(i, q)
            # single Z evacuation at the very end
            stgZ = stg_p.tile([128, 512], f32, tag="stgZ", bufs=1, name="stgZ")
            nc.vector.tensor_copy(out=stgZ, in_=accZ)
            nc.sync.dma_start(out=outZ_d.ap(), in_=stgZ)
    nc.compile()
    _NC_CACHE["nc"] = nc
    return nc


def _host_prep(G_queries, G_keys, G_values, L_values, sigma):
    """Build per-core input dicts + host-side encg [L, H] per core."""
    inv_sqrt_2pi = 1.0 / math.sqrt(2.0 * math.pi)
    sig = sigma.astype(np.float32)
    sig = 1.0 / (1.0 + np.exp(-5.0 * sig.astype(np.float64)))
    sig = (sig + 1e-05).astype(np.float32)
    sig = (np.float32(3.0) ** sig) - np.float32(1.0)          # [B, L, H]
    c = inv_sqrt_2pi / sig.astype(np.float64)                  # [B, L, H]
    encg = np.exp(-c)                                          # [B, L, H]
    nhi = 1.0 / (2.0 * sig.astype(np.float64) ** 2)

    in_maps = []
    aux = []
    for b in range(B):
        qkt = np.empty((4, 128, 2 * L), BF16)
        for h in range(H):
            i, p = divmod(h, 2)
            qkt[i, 64 * p:64 * p + 64, :L] = G_queries[b, :, h, :].T
            qkt[i, 64 * p:64 * p + 64, L:] = G_keys[b, :, h, :].T
        # packed V: [V_g | V_l | ones] per chunk, layout [H, 128, NCH, 129]
        vgl = np.empty((H, 128, NCH, 129), BF16)
        gv = G_values[b].reshape(NCH, 128, H, E)   # [k, p, h, e]
        lv = L_values[b].reshape(NCH, 128, H, E)
        vgl[:, :, :, 0:64] = np.ascontiguousarray(gv.transpose(2, 1, 0, 3))
        vgl[:, :, :, 64:128] = np.ascontiguousarray(lv.transpose(2, 1, 0, 3))
        vgl[..., 128] = 1.0
        # W band tiles [4, 128, NCH, 2, WW] (pairs of heads)
        wband = np.zeros((4, 128, NCH, 2, WW), BF16)
        s_off = np.arange(128)
        j_off = np.arange(WW)
        for k in range(NCH):
            s_idx = 128 * k + s_off                  # [128]
            l_idx = 128 * k - BAND + j_off           # [WW]
            valid = (l_idx >= 0) & (l_idx < L)
            lvx = np.clip(l_idx, 0, L - 1)
            d = l_idx[None, :] - s_idx[:, None]      # [128, WW]
            band_ok = (np.abs(d) <= BAND) & valid[None, :]
            for h in range(H):
                i, p = divmod(h, 2)
                ch = c[b, lvx, h][None, :]
                g = ch * np.exp(-(d.astype(np.float64) ** 2) * nhi[b, lvx, h][None, :])
                W = np.exp(g - ch) - encg[b, lvx, h][None, :]
                W[~band_ok] = 0.0
                wband[i, :, k, p, :] = W.astype(np.float32)
        in_maps.append({"qkt": np.asarray(qkt),
                        "vgl": np.asarray(vgl),
                        "wband": np.asarray(wband)})
        aux.append(encg[b])  # [L, H]
    return in_maps, aux


def _host_post(outs, aux):
    G_V = np.empty((B, L, H, E), np.float32)
    L_V = np.empty((B, L, H, E), np.float32)
    for b in range(B):
        oGL = outs[b]["outGL"].astype(np.float64)  # [H, 2, 128, 512]
        oB = outs[b]["outB"].astype(np.float64)    # [H, 2, 65, 512]
        oZ = outs[b]["outZ"].astype(np.float64)    # [128, 512] one-hot packed
        # Z for (h, half) = m = 4i+2p+half: sum of the 4 col-group streams
        zm = oZ[0:16] + oZ[32:48] + oZ[64:80] + oZ[96:112]   # [16, 512]
        for h in range(H):
            i, p = divmod(h, 2)
            GLt = np.concatenate([oGL[h, 0], oGL[h, 1]], axis=1)  # [128, L]
            Bt = np.concatenate([oB[h, 0], oB[h, 1]], axis=1)     # [65, L]
            Z = np.concatenate([zm[4 * i + 2 * p + 0],
                                zm[4 * i + 2 * p + 1]])           # [L]
            e = aux[b][:, h]  # [L]
            G_V[b, :, h, :] = (GLt[0:64] / Z).T
            Lnum = GLt[64:128] * e[None, :] + Bt[:64]
            Lden = Z * e + Bt[64]
            L_V[b, :, h, :] = (Lnum / Lden).T
    return G_V, L_V


def kernel(G_queries, G_keys, G_values, L_values, sigma):
    from concourse.bass_utils import run_bass_kernel_spmd

    args = [np.asarray(x, dtype=np.float32) for x in
            (G_queries, G_keys, G_values, L_values, sigma)]
    nc = _build_nc()
    in_maps, aux = _host_prep(*args)
    res = run_bass_kernel_spmd(nc, in_maps, core_ids=list(range(N_CORES)),
                               trace=False)
    return _host_post(res.results, aux)


# revision 14
# speedup vs baseline: 1.0411x; 1.0051x over previous
"""AnomalyAttention on 8 Trainium2 NeuronCores (Bass/Tile), data-parallel over batch.

Problem: B,L,H,E = 8,1024,8,64
  score  = (1/sqrt(E)) * einsum('blhe,bshe->bhls', Q, K)
  gauss  = kappa/sig_l * exp(-(l-s)^2 / (2 sig_l^2))       (kappa = 1/sqrt(2 pi))
  G_V    = softmax(score, s) @ G_values
  L_V    = softmax(score + gauss, s) @ L_values

Device strategy (per core = one batch element, loop over 8 heads):
  Work in the transposed layout P[s, l] so the attention@V matmuls need no
  transposes.  With c_l = gauss[l,l] (peak), exp(score+gauss-c_l) =
  P * exp(gauss-c_l) where P = exp(score) is shared with the Global path and
  exp(gauss-c_l) equals the constant encg_l = exp(-c_l) except in a +-16 band
  around the diagonal.  So:
    L_num   = encg_l * (Vl^T P) + Vl^T (P .* W)     with W = exp(gauss-c)-encg
    L_den   = encg_l * Z + ones^T (P .* W),  Z = ones^T P
  The device computes  Vg^T P | Vl^T P (one fused M=128 matmul chain),
  Vl^T (P.*W) (banded, with a ones column for the band Z row), and Z = 1^T P.

  Engine balance: the L*L exp is split between ScalarE (true exp, ~2/3 of
  chunks) and VectorE (single-op Schraudolph fast-exp: i16 = round(a*score+b)
  bit-viewed as bf16, +-3% rel err on 3/8 of chunks).  Z uses one persistent
  PSUM bank with one-hot M=16 weights per (head, half) in 4 col-group-packed
  streams, so only ONE final Z copy/DMA is needed.  Band multiplies cover both
  heads of a pair per instruction ([128, 2, len] APs).  PSUM evacuation is
  split: GL tiles on VectorE, B tiles on ScalarE.
"""

import math
import numpy as np
import ml_dtypes

BF16 = ml_dtypes.bfloat16
B, L, H, E = 8, 1024, 8, 64
NCH = L // 128          # 8 s-chunks of 128
BAND = 16               # gauss band halfwidth (W < 3e-7 beyond; bf16-invisible)
WW = 128 + 2 * BAND     # 160: W tile width in l per s-chunk
N_CORES = 8
DVE_KS = (1, 3, 5)      # s-chunks whose exp runs on VectorE (fast-exp)

LOG2E = 1.4426950408889634
FX_SIGMA = 0.0579       # Schraudolph minimax centering
FX_A = 128.0 * LOG2E / math.sqrt(E)       # folds the 1/sqrt(E) score scale
FX_B = (127.0 - FX_SIGMA) * 128.0

_NC_CACHE = {}


def _build_nc():
    if "nc" in _NC_CACHE:
        return _NC_CACHE["nc"]
    import concourse.bacc as bacc
    import concourse.tile as tile
    from concourse import mybir
    from concourse.tile import add_dep_helper

    f32 = mybir.dt.float32
    bf16 = mybir.dt.bfloat16
    i16 = mybir.dt.int16

    nc = bacc.Bacc()
    qkt_d = nc.declare_dram_parameter("qkt", [4, 128, 2 * L], bf16, isOutput=False)
    # vgl[h, :, k, 0:64] = V_g chunk, 64:128 = V_l chunk, col 128 = ones
    vgl_d = nc.declare_dram_parameter("vgl", [H, 128, NCH, 129], bf16, isOutput=False)
    # wband2[i, :, k, p, :]: gauss band W for head 2i+p
    wbd_d = nc.declare_dram_parameter("wband", [4, 128, NCH, 2, WW], bf16,
                                      isOutput=False)
    # out[h, half, 0] = [128, 512]: rows 0:64 Vg^T P, rows 64:128 Vl^T P
    # out[h, half, 1] = [65, 512]: band correction (+ band Z row 64)
    outGL_d = nc.declare_dram_parameter("outGL", [H, 2, 128, 512], bf16, isOutput=True)
    outB_d = nc.declare_dram_parameter("outB", [H, 2, 65, 512], bf16, isOutput=True)
    # one-hot Z accumulator: row 32j+m = partial Z for (h,half)=m over chunks
    # k == j (mod 4); host sums the 4 j-streams.
    outZ_d = nc.declare_dram_parameter("outZ", [128, 512], f32, isOutput=True)

    with tile.TileContext(nc) as tc:
        with (
            tc.tile_pool(name="ones_p", bufs=1) as ones_p,
            tc.tile_pool(name="qkt_p", bufs=2) as qkt_p,
            tc.tile_pool(name="v_p", bufs=4) as v_p,
            tc.tile_pool(name="w_p", bufs=2) as w_p,
            tc.tile_pool(name="pg_p", bufs=20) as pg_p,
            tc.tile_pool(name="mb_p", bufs=34) as mb_p,
            tc.tile_pool(name="stg_p", bufs=3) as stg_p,
            tc.tile_pool(name="sc_p", bufs=2, space="PSUM") as sc_p,
            tc.tile_pool(name="acc_p", bufs=1, space="PSUM") as acc_p,
        ):
            # sliding one-hot weight bank for Z: col 15 is ones; variant m
            # (= 4i+2p+half) is zw[:, 15-m:31-m]
            zw = ones_p.tile([128, 31], bf16, tag="zw", bufs=1)
            nc.vector.memset(zw, 0.0)
            nc.vector.memset(zw[:, 15:16], 1.0)
            zrow = ones_p.tile([1, 512], bf16, tag="zrow", bufs=1)
            nc.vector.memset(zrow, 0.0)
            # one persistent Z bank for the whole kernel; cleared once, all Z
            # matmuls start=False; single final copy+DMA
            accZ = acc_p.tile([128, 512], f32, tag="accZ", bufs=1)
            nc.tensor.matmul(out=accZ, lhsT=zrow[:, 0:128], rhs=zrow,
                             start=True, stop=False, skip_group_check=True)

            state = {}

            def emit_sweep_half(i, q):
                if q == 0:
                    qt = qkt_p.tile([128, 2 * L], bf16, tag="qkt", bufs=2,
                                    name="qt")
                    # DMA in need-order: keys-lo + queries-q0 unblock the
                    # first QK chunks; keys-hi next; queries-q1 in half 1.
                    nc.sync.dma_start(out=qt[:, 1024:1536],
                                      in_=qkt_d.ap()[i][:, 1024:1536])
                    nc.sync.dma_start(out=qt[:, 0:512],
                                      in_=qkt_d.ap()[i][:, 0:512])
                    nc.sync.dma_start(out=qt[:, 1536:2048],
                                      in_=qkt_d.ap()[i][:, 1536:2048])
                    wb = w_p.tile([128, NCH, 2, WW], bf16, tag="wb", bufs=2,
                                  name="wb")
                    nc.scalar.dma_start(out=wb, in_=wbd_d.ap()[i])
                    state[i] = {
                        "qt": qt, "wb": wb, "mb": [None] * NCH,
                        "pgq": [[None, None] for _ in range(NCH)],
                        "vgls": [],
                    }
                else:
                    qt = state[i]["qt"]
                    wb = state[i]["wb"]
                    nc.sync.dma_start(out=qt[:, 512:1024],
                                      in_=qkt_d.ap()[i][:, 512:1024])
                st = state[i]
                # QK with both heads in lockstep: per (chunk k, l-half q), one
                # [128, 2, 512] PSUM tile holds head-e scores in [:,0,:] and
                # head-o scores in [:,1,:]; the two matmuls use disjoint PE row
                # groups.  exp: ScalarE true exp or VectorE fast-exp per chunk.
                for k in range(NCH):
                    sc = sc_p.tile([128, 2, 512], f32, tag="sc", bufs=2,
                                   name="sc")
                    for p in range(2):
                        pslc = slice(64 * p, 64 * p + 64)
                        nc.tensor.matmul(
                            out=sc[:, p, :],
                            lhsT=qt[pslc, L + 128 * k:L + 128 * (k + 1)],
                            rhs=qt[pslc, 512 * q:512 * (q + 1)],
                            start=True, stop=True,
                            tile_position=(64 * p, 0),
                        )
                    pgk = pg_p.tile([128, 2, 512], bf16, tag="pg", bufs=40,
                                    name="pgk")
                    if k in DVE_KS:
                        nc.vector.tensor_scalar(
                            out=pgk.bitcast(i16), in0=sc,
                            scalar1=FX_A, scalar2=FX_B,
                            op0=mybir.AluOpType.mult,
                            op1=mybir.AluOpType.add,
                        )
                    else:
                        nc.scalar.activation(
                            out=pgk, in_=sc,
                            func=mybir.ActivationFunctionType.Exp,
                            scale=1.0 / math.sqrt(E),
                        )
                    st["pgq"][k][q] = pgk
                    # band product piece for this l-half (both heads at once)
                    a0 = max(0, 128 * k - BAND)
                    b0 = min(L, 128 * k + 128 + BAND)
                    a = max(a0, 512 * q)
                    bb = min(b0, 512 * (q + 1))
                    if bb > a:
                        if st["mb"][k] is None:
                            st["mb"][k] = mb_p.tile([128, 2, WW], bf16,
                                                    tag="mb", bufs=24,
                                                    name="mbk")
                        woff = a - (128 * k - BAND)
                        nc.vector.tensor_mul(
                            out=st["mb"][k][:, :, woff:woff + (bb - a)],
                            in0=pgk[:, :, a - 512 * q:bb - 512 * q],
                            in1=wb[:, k, :, woff:woff + (bb - a)],
                        )
                if q == 0:
                    for p in range(2):
                        h = 2 * i + p
                        vgl = v_p.tile([128, NCH, 129], bf16, tag=f"vgl{p}",
                                       bufs=2, name="vgl")
                        nc.scalar.dma_start(out=vgl, in_=vgl_d.ap()[h])
                        st["vgls"].append(vgl)

            def emit_av_half(i, half):
                st = state[i]
                pgq, mb = st["pgq"], st["mb"]
                for p in range(2):
                    h = 2 * i + p
                    vgl = st["vgls"][p]
                    m = 4 * i + 2 * p + half
                    h0 = half * 512
                    accGL = acc_p.tile([128, 512], f32, tag="accGL", bufs=2,
                                       name="accGL")
                    accB = acc_p.tile([65, 512], f32, tag="accB", bufs=1,
                                      name="accB")
                    for k in range(NCH):
                        rhs = pgq[k][half][:, p, :]
                        nc.tensor.matmul(out=accGL, lhsT=vgl[:, k, 0:128],
                                         rhs=rhs,
                                         start=(k == 0), stop=(k == NCH - 1))
                    for k in range(NCH):
                        rhs = pgq[k][half][:, p, :]
                        j = k % 4
                        nc.tensor.matmul(out=accZ[32 * j:32 * j + 16, :],
                                         lhsT=zw[:, 15 - m:31 - m], rhs=rhs,
                                         start=False, stop=False,
                                         tile_position=(0, 32 * j),
                                         skip_group_check=True)
                    spans = []
                    for k in range(NCH):
                        a = max(0, 128 * k - BAND, h0)
                        bb = min(L, 128 * k + 128 + BAND, h0 + 512)
                        if bb > a:
                            spans.append((k, a, bb))
                    b_first = None
                    for jj, (k, a, bb) in enumerate(spans):
                        off = a - (128 * k - BAND)
                        mmb = nc.tensor.matmul(
                            out=accB[:, a - h0:bb - h0],
                            lhsT=vgl[:, k, 64:129],
                            rhs=mb[k][:, p, off:off + (bb - a)],
                            start=(jj == 0), stop=(jj == len(spans) - 1),
                            skip_group_check=True,
                        )
                        if jj == 0:
                            b_first = mmb
                        else:
                            add_dep_helper(mmb.ins, b_first.ins,
                                           reason="bank clear first")
                    stgGL = stg_p.tile([128, 512], bf16, tag="stgGL", bufs=3,
                                       name="stgGL")
                    nc.vector.tensor_copy(out=stgGL, in_=accGL)
                    nc.sync.dma_start(out=outGL_d.ap()[h, half], in_=stgGL)
                    stgB = stg_p.tile([65, 512], bf16, tag="stgB", bufs=3,
                                      name="stgB")
                    nc.scalar.copy(out=stgB, in_=accB)
                    nc.scalar.dma_start(out=outB_d.ap()[h, half], in_=stgB)

            # software pipeline at half-sweep granularity: AV blocks for
            # (pair, half) are emitted one half-step behind the QK/exp sweep
            # so the final PE-only tail is a single av-half, not a full pair.
            steps = []
            for i in range(4):
                steps.append(("S", i, 0))
                steps.append(("S", i, 1))
            sched = []
            av_q = []
            for kind, i, q in steps:
                sched.append(("S", i, q))
                av_q.append(("A", i, q))
                if len(av_q) >= 2:
                    sched.append(av_q.pop(0))
            sched.extend(av_q)
            for kind, i, q in sched:
                if kind == "S":
                    emit_sweep_half(i, q)
                else:
                    emit_av_half(i, q)
            # single Z evacuation at the very end
            stgZ = stg_p.tile([128, 512], f32, tag="stgZ", bufs=1, name="stgZ")
            nc.vector.tensor_copy(out=stgZ, in_=accZ)
            nc.sync.dma_start(out=outZ_d.ap(), in_=stgZ)
    nc.compile()
    _NC_CACHE["nc"] = nc
    return nc


def _host_prep(G_queries, G_keys, G_values, L_values, sigma):
    """Build per-core input dicts + host-side encg [L, H] per core."""
    inv_sqrt_2pi = 1.0 / math.sqrt(2.0 * math.pi)
    sig = sigma.astype(np.float32)
    sig = 1.0 / (1.0 + np.exp(-5.0 * sig.astype(np.float64)))
    sig = (sig + 1e-05).astype(np.float32)
    sig = (np.float32(3.0) ** sig) - np.float32(1.0)          # [B, L, H]
    c = inv_sqrt_2pi / sig.astype(np.float64)                  # [B, L, H]
    encg = np.exp(-c)                                          # [B, L, H]
    nhi = 1.0 / (2.0 * sig.astype(np.float64) ** 2)

    in_maps = []
    aux = []
    for b in range(B):
        qkt = np.empty((4, 128, 2 * L), BF16)
        for h in range(H):
            i, p = divmod(h, 2)
            qkt[i, 64 * p:64 * p + 64, :L] = G_queries[b, :, h, :].T
            qkt[i, 64 * p:64 * p + 64, L:] = G_keys[b, :, h, :].T
        # packed V: [V_g | V_l | ones] per chunk, layout [H, 128, NCH, 129]
        vgl = np.empty((H, 128, NCH, 129), BF16)
        gv = G_values[b].reshape(NCH, 128, H, E)   # [k, p, h, e]
        lv = L_values[b].reshape(NCH, 128, H, E)
        vgl[:, :, :, 0:64] = np.ascontiguousarray(gv.transpose(2, 1, 0, 3))
        vgl[:, :, :, 64:128] = np.ascontiguousarray(lv.transpose(2, 1, 0, 3))
        vgl[..., 128] = 1.0
        # W band tiles [4, 128, NCH, 2, WW] (pairs of heads)
        wband = np.zeros((4, 128, NCH, 2, WW), BF16)
        s_off = np.arange(128)
        j_off = np.arange(WW)
        for k in range(NCH):
            s_idx = 128 * k + s_off                  # [128]
            l_idx = 128 * k - BAND + j_off           # [WW]
            valid = (l_idx >= 0) & (l_idx < L)
            lvx = np.clip(l_idx, 0, L - 1)
            d = l_idx[None, :] - s_idx[:, None]      # [128, WW]
            band_ok = (np.abs(d) <= BAND) & valid[None, :]
            for h in range(H):
                i, p = divmod(h, 2)
                ch = c[b, lvx, h][None, :]
                g = ch * np.exp(-(d.astype(np.float64) ** 2) * nhi[b, lvx, h][None, :])
                W = np.exp(g - ch) - encg[b, lvx, h][None, :]
                W[~band_ok] = 0.0
                wband[i, :, k, p, :] = W.astype(np.float32)
        in_maps.append({"qkt": np.asarray(qkt),
                        "vgl": np.asarray(vgl),
                        "wband": np.asarray(wband)})
        aux.append(encg[b])  # [L, H]
    return in_maps, aux


def _host_post(outs, aux):
    G_V = np.empty((B, L, H, E), np.float32)
    L_V = np.empty((B, L, H, E), np.float32)
    for b in range(B):
        oGL = outs[b]["outGL"].astype(np.float64)  # [H, 2, 128, 512]
        oB = outs[b]["outB"].astype(np.float64)    # [H, 2, 65, 512]
        oZ = outs[b]["outZ"].astype(np.float64)    # [128, 512] one-hot packed
        # Z for (h, half) = m = 4i+2p+half: sum of the 4 col-group streams
        zm = oZ[0:16] + oZ[32:48] + oZ[64:80] + oZ[96:112]   # [16, 512]
        for h in range(H):
            i, p = divmod(h, 2)
            GLt = np.concatenate([oGL[h, 0], oGL[h, 1]], axis=1)  # [128, L]
            Bt = np.concatenate([oB[h, 0], oB[h, 1]], axis=1)     # [65, L]
            Z = np.concatenate([zm[4 * i + 2 * p + 0],
                                zm[4 * i + 2 * p + 1]])           # [L]
            e = aux[b][:, h]  # [L]
            G_V[b, :, h, :] = (GLt[0:64] / Z).T
            Lnum = GLt[64:128] * e[None, :] + Bt[:64]
            Lden = Z * e + Bt[64]
            L_V[b, :, h, :] = (Lnum / Lden).T
    return G_V, L_V


def kernel(G_queries, G_keys, G_values, L_values, sigma):
    from concourse.bass_utils import run_bass_kernel_spmd

    args = [np.asarray(x, dtype=np.float32) for x in
            (G_queries, G_keys, G_values, L_values, sigma)]
    nc = _build_nc()
    in_maps, aux = _host_prep(*args)
    res = run_bass_kernel_spmd(nc, in_maps, core_ids=list(range(N_CORES)),
                               trace=False)
    return _host_post(res.results, aux)
